# revision 2
# baseline (speedup 1.0000x reference)
"""Trainium2 Bass kernel for nn_CenterCrop: per-sample resize(short-side=256)
+ center-crop(224), bilinear, batch sharded over 8 NeuronCores.

Bilinear resize is separable: out = S^T @ img @ G with per-sample sparse
interpolation matrices S (vertical) and G (horizontal), built on the host
from the h/w metadata. The gather+lerp itself runs on the PE array as fp32
matmuls (exact):
  pass1: tmp1_T[x, j] = sum_y img[y, x] * S[y, j]   (img tiles stationary)
  pass2: out[j, i]    = sum_x tmp1_T[x, j] * G[x, i] (tmp1 tiles stationary)

Perf structure:
- Only the per-sample source window [y0min:y1max, x0min:x1max] that the
  output actually reads (~0.875*min(h,w) squared, 35-80% of the image) is
  DMA'd and processed.
- S/G are banded: each 128-row K-tile only touches a narrow output column
  range, so every matmul streams just that band (PSUM has_written bits make
  split accumulation exact).
- SPMD requires one program for all 8 cores, so samples are sorted by
  min(h,w) and dealt round-robin: slot s on every core holds same-sized
  windows; the program is specialized per-slot to the union shape/bands.
  Outputs are unpermuted on the host.

Measured on trn2 (8 cores): HW exec 115.2 us, max rel err 1.6e-7
(naive full-image fp32 matmul version: 245 us). PE-bound at the fp32
instruction floor (~324 logical matmuls x 2 LDW+MM pairs x ~150 ns);
DMA is 20.6 MB/core (~58 us at roofline).
"""

import sys
import os

for _p in ("/opt/trn_rl_repo",):
    if os.path.isdir(_p) and _p not in sys.path:
        sys.path.insert(0, _p)

import numpy as np

OUT_H = 224
OUT_W = 224
RESIZE_TO = np.float32(256.0)
B_FULL = 64
N_CORES = 8
B_LOC = B_FULL // N_CORES  # 8 slots per core
C = 3
H = 512
W = 512  # image width after stripping the metadata column (stored width 513)

LAST_EXEC_NS = None
LAST_RESULTS = None
_NC_CACHE = {}

# float32r experiment: single-pass PE fp32 (TF32-like rate) with streams
# padded to 256 cols to hit the fast path. Gated off by default; flip via
# env CENTERCROP_F32R=1 to measure precision/speed.
USE_F32R = os.environ.get("CENTERCROP_F32R", "0") == "1"
PADN = 256  # padded stream width under f32r

# bf16 3-pass experiment: img = A+B and S = Sa+Sb as exact bf16 pairs;
# img@S ~= A@Sa + B@Sa + A@Sb (dropped B@Sb <= 2^-18) -> ~1e-5 rel err
# with 1cyc/row streams and FWL weight loads.
USE_BF16 = os.environ.get("CENTERCROP_BF16", "0") == "1"

# bf16 single-pass (default): cast img/S/G to bf16, one matmul per tile,
# fp32 PSUM accumulate. ~5e-3 rel err vs the 2e-2 gate; full-rate PE with
# FWL weight loads and half the DMA bytes of fp32.
USE_BF16_1P = (os.environ.get("CENTERCROP_F32", "0") != "1"
               and not USE_F32R and not USE_BF16)


def _interp_matrices(h, w):
    """Full S [512, OUT_H], G [512, OUT_W] fp32 interpolation matrices,
    mirroring the reference fp32 math bit-for-bit."""
    f32 = np.float32
    h = f32(h)
    w = f32(w)
    min_dim = min(h, w)
    scale = RESIZE_TO / min_dim
    h_res = np.round(h * scale)
    w_res = np.round(w * scale)
    top = np.round((h_res - f32(OUT_H)) / f32(2.0))
    left = np.round((w_res - f32(OUT_W)) / f32(2.0))

    def axis_mat(n_out, offset, dim, dim_res, n_src):
        idx = np.arange(n_out, dtype=np.float32) + offset
        src = np.clip((idx + f32(0.5)) * dim / dim_res - f32(0.5),
                      f32(0.0), dim - f32(1.0))
        p0f = np.floor(src)
        frac = src - p0f
        imax = np.int32(dim) - 1
        p0 = np.clip(p0f.astype(np.int32), 0, imax)
        p1 = np.minimum(p0 + 1, imax)
        mat = np.zeros((n_src, n_out), np.float32)
        cols = np.arange(n_out)
        np.add.at(mat, (p0, cols), f32(1.0) - frac)
        np.add.at(mat, (p1, cols), frac)
        return mat

    S = axis_mat(OUT_H, top, h, h_res, H)
    G = axis_mat(OUT_W, left, w, w_res, W)
    return S, G


def _bands(mat_w, n_tiles):
    """Per-128-row-tile [lo, hi) columns with any nonzero; None if empty."""
    out = []
    for t in range(n_tiles):
        rows = mat_w[t * 128:(t + 1) * 128]
        nz = np.nonzero(rows.any(axis=0))[0]
        out.append(None if nz.size == 0 else (int(nz[0]), int(nz[-1]) + 1))
    return out


def _union_bands(band_lists):
    n = len(band_lists[0])
    out = []
    for t in range(n):
        los = [b[t][0] for b in band_lists if b[t] is not None]
        his = [b[t][1] for b in band_lists if b[t] is not None]
        out.append(None if not los else (min(los), max(his)))
    return out


def _prepare(x):
    """Host prep: per-sample windows/matrices, sorted slot assignment,
    per-core packed inputs, and the per-slot program parameters."""
    h_all = x[:, 0, 0, -1].astype(np.float32)
    w_all = x[:, 1, 0, -1].astype(np.float32)

    samples = []
    for b in range(B_FULL):
        S, G = _interp_matrices(h_all[b], w_all[b])
        ynz = np.nonzero(S.any(axis=1))[0]
        xnz = np.nonzero(G.any(axis=1))[0]
        y0, y1 = int(ynz[0]), int(ynz[-1]) + 1
        x0, x1 = int(xnz[0]), int(xnz[-1]) + 1
        samples.append(dict(S=S[y0:y1], G=G[x0:x1], y0=y0, x0=x0,
                            wh=y1 - y0, ww=x1 - x0))

    order = np.argsort(np.minimum(h_all, w_all), kind="stable")
    # slot s, core c -> sample order[s*N_CORES + c]
    assign = [[int(order[s * N_CORES + c]) for c in range(N_CORES)]
              for s in range(B_LOC)]

    slot_params = []
    slot_data = []  # per slot: list over cores of (sid, Sw_pad, Gw_pad)
    for s in range(B_LOC):
        sids = assign[s]
        wh = max(samples[i]["wh"] for i in sids)
        ww = max(samples[i]["ww"] for i in sids)
        n_yt = (wh + 127) // 128
        n_xt = (ww + 127) // 128
        sb_list, gb_list, data = [], [], []
        for i in sids:
            sp = samples[i]
            Sw = np.zeros((n_yt * 128, OUT_H), np.float32)
            Sw[:sp["wh"]] = sp["S"]
            Gw = np.zeros((n_xt * 128, OUT_W), np.float32)
            Gw[:sp["ww"]] = sp["G"]
            sb_list.append(_bands(Sw, n_yt))
            gb_list.append(_bands(Gw, n_xt))
            data.append((i, Sw, Gw))
        sbands = _union_bands(sb_list)
        gbands = _union_bands(gb_list)
        slot_params.append((n_yt, n_xt, ww,
                            tuple(sbands), tuple(gbands)))
        slot_data.append(data)

    # pack per-core input maps
    sgw = PADN if USE_F32R else OUT_H
    in_maps = [{} for _ in range(N_CORES)]
    for s in range(B_LOC):
        n_yt, n_xt, ww, _, _ = slot_params[s]
        for c in range(N_CORES):
            sid, Sw, Gw = slot_data[s][c]
            sp = samples[sid]
            xw = np.zeros((C, n_yt * 128, ww), np.float32)
            xw[:, :sp["wh"], :sp["ww"]] = x[
                sid, :, sp["y0"]:sp["y0"] + sp["wh"],
                sp["x0"]:sp["x0"] + sp["ww"]]
            st = np.zeros((128, n_yt, sgw), np.float32)
            st[:, :, :OUT_H] = Sw.reshape(n_yt, 128, OUT_H).transpose(1, 0, 2)
            gt = np.zeros((128, n_xt, sgw), np.float32)
            gt[:, :, :OUT_W] = Gw.reshape(n_xt, 128, OUT_W).transpose(1, 0, 2)
            if USE_BF16:
                import ml_dtypes
                bf16 = ml_dtypes.bfloat16
                for nm, arr in (("xw", xw), ("s", st), ("g", gt)):
                    a = arr.astype(bf16)
                    b = (arr - a.astype(np.float32)).astype(bf16)
                    in_maps[c][f"{nm}a{s}"] = a
                    in_maps[c][f"{nm}b{s}"] = b
            else:
                in_maps[c][f"xw{s}"] = xw
                in_maps[c][f"s{s}"] = st
                in_maps[c][f"g{s}"] = gt
    return tuple(slot_params), in_maps, assign


def _build_nc(slot_params):
    import concourse.bacc as bacc
    import concourse.mybir as mybir
    import concourse.tile as tile

    dt = mybir.dt.float32
    dtd = mybir.dt.float32r if USE_F32R else mybir.dt.float32
    sgw = PADN if USE_F32R else OUT_H
    nc = bacc.Bacc(
        "TRN2",
        target_bir_lowering=False,
        debug=False,
        enable_asserts=False,
        num_devices=N_CORES,
    )
    dtb = mybir.dt.bfloat16
    if USE_BF16:
        dtd = dtb
    xw_in, s_in, g_in = [], [], []
    for s, (n_yt, n_xt, ww, _, _) in enumerate(slot_params):
        if USE_BF16:
            xw_in.append(tuple(
                nc.dram_tensor(f"xw{h}{s}", [C, n_yt * 128, ww], dtb,
                               kind="ExternalInput") for h in "ab"))
            s_in.append(tuple(
                nc.dram_tensor(f"s{h}{s}", [128, n_yt, sgw], dtb,
                               kind="ExternalInput") for h in "ab"))
            g_in.append(tuple(
                nc.dram_tensor(f"g{h}{s}", [128, n_xt, sgw], dtb,
                               kind="ExternalInput") for h in "ab"))
        else:
            xw_in.append(nc.dram_tensor(f"xw{s}", [C, n_yt * 128, ww], dtd,
                                        kind="ExternalInput"))
            s_in.append(nc.dram_tensor(f"s{s}", [128, n_yt, sgw], dtd,
                                       kind="ExternalInput"))
            g_in.append(nc.dram_tensor(f"g{s}", [128, n_xt, sgw], dtd,
                                       kind="ExternalInput"))
    out = nc.dram_tensor("out", [B_LOC, C, OUT_H, OUT_W], dt,
                         kind="ExternalOutput")

    # ascending size: the first compute wave waits on the smallest DMA
    slot_order = list(range(len(slot_params)))

    with tile.TileContext(nc) as tc:
        with (
            tc.tile_pool(name="img", bufs=4) as img_pool,
            tc.tile_pool(name="sg", bufs=3) as sg_pool,
            tc.tile_pool(name="tmp", bufs=2) as tmp_pool,
            tc.tile_pool(name="outp", bufs=2) as out_pool,
            tc.tile_pool(name="ps1", bufs=3, space="PSUM") as ps1_pool,
            tc.tile_pool(name="ps2", bufs=3, space="PSUM") as ps2_pool,
        ):
            for s in slot_order:
                n_yt, n_xt, ww, sbands, gbands = slot_params[s]
                if USE_BF16:
                    s_sb = [sg_pool.tile([128, n_yt, sgw], dtb, tag=f"s{h}",
                                          name=f"s{h}_sb")
                            for h in "ab"]
                    g_sb = [sg_pool.tile([128, n_xt, sgw], dtb, tag=f"g{h}",
                                          name=f"g{h}_sb")
                            for h in "ab"]
                    for h in range(2):
                        nc.sync.dma_start(s_sb[h][:], s_in[s][h][:])
                        nc.sync.dma_start(g_sb[h][:], g_in[s][h][:])
                else:
                    s_sb = sg_pool.tile([128, n_yt, sgw], dtd, tag="s")
                    g_sb = sg_pool.tile([128, n_xt, sgw], dtd, tag="g")
                    nc.sync.dma_start(s_sb[:], s_in[s][:])
                    nc.sync.dma_start(g_sb[:], g_in[s][:])
                out_sb = out_pool.tile([112, C, 2, OUT_W], dt)
                s_emit = [t for t in range(n_yt) if sbands[t] is not None]
                g_emit = [t for t in range(n_xt) if gbands[t] is not None]
                for c in range(C):
                    if USE_BF16:
                        img_sb = [img_pool.tile([128, n_yt, ww], dtb, tag=f"img{h}",
                                                name=f"img{h}_sb")
                                  for h in "ab"]
                        for h in range(2):
                            src = xw_in[s][h][c].rearrange(
                                "(t p) x -> p t x", p=128)
                            nc.sync.dma_start(img_sb[h][:], src)
                        tmp_sb = [tmp_pool.tile([128, n_xt, OUT_H], dtb, tag=f"tmp{h}",
                                                name=f"tmp{h}_sb")
                                  for h in "ab"]
                    else:
                        img_sb = img_pool.tile([128, n_yt, ww], dtd)
                        src = xw_in[s][c].rearrange("(t p) x -> p t x", p=128)
                        nc.sync.dma_start(img_sb[:], src)
                        tmp_sb = tmp_pool.tile([128, n_xt, OUT_H], dtd)
                    for xb in range(n_xt):
                        xlo = xb * 128
                        xn = min(128, ww - xlo)
                        ps1 = ps1_pool.tile([128, sgw], dt)
                        if USE_BF16:
                            n3 = 3 * len(s_emit)
                            k = 0
                            for t in s_emit:
                                lo, hi = sbands[t]
                                for iw, sw in ((0, 0), (1, 0), (0, 1)):
                                    nc.tensor.matmul(
                                        ps1[:xn, lo:hi],
                                        img_sb[iw][:, t, xlo:xlo + xn],
                                        s_sb[sw][:, t, lo:hi],
                                        start=(k == 0),
                                        stop=(k == n3 - 1),
                                        skip_group_check=True,
                                    )
                                    k += 1
                            nc.vector.tensor_copy(tmp_sb[0][:xn, xb, :],
                                                  ps1[:xn, :OUT_H])
                            nc.vector.tensor_sub(tmp_sb[1][:xn, xb, :],
                                                 ps1[:xn, :OUT_H],
                                                 tmp_sb[0][:xn, xb, :])
                        else:
                            for i_t, t in enumerate(s_emit):
                                lo, hi = (0, sgw) if USE_F32R else sbands[t]
                                nc.tensor.matmul(
                                    ps1[:xn, lo:hi],
                                    img_sb[:, t, xlo:xlo + xn],
                                    s_sb[:, t, lo:hi],
                                    start=(i_t == 0),
                                    stop=(i_t == len(s_emit) - 1),
                                    skip_group_check=True,
                                )
                            nc.vector.tensor_copy(tmp_sb[:xn, xb, :],
                                                  ps1[:xn, :OUT_H])
                    for jb in range(2):
                        ps2 = ps2_pool.tile([112, sgw], dt)
                        if USE_BF16:
                            n3 = 3 * len(g_emit)
                            k = 0
                            for xb in g_emit:
                                lo, hi = gbands[xb]
                                xn = min(128, ww - xb * 128)
                                for tw, gw in ((0, 0), (1, 0), (0, 1)):
                                    nc.tensor.matmul(
                                        ps2[:, lo:hi],
                                        tmp_sb[tw][:xn, xb,
                                                   jb * 112:(jb + 1) * 112],
                                        g_sb[gw][:xn, xb, lo:hi],
                                        start=(k == 0),
                                        stop=(k == n3 - 1),
                                        skip_group_check=True,
                                    )
                                    k += 1
                        else:
                            for i_t, xb in enumerate(g_emit):
                                lo, hi = (0, sgw) if USE_F32R else gbands[xb]
                                xn = min(128, ww - xb * 128)
                                nc.tensor.matmul(
                                    ps2[:, lo:hi],
                                    tmp_sb[:xn, xb, jb * 112:(jb + 1) * 112],
                                    g_sb[:xn, xb, lo:hi],
                                    start=(i_t == 0),
                                    stop=(i_t == len(g_emit) - 1),
                                    skip_group_check=True,
                                )
                        nc.vector.tensor_copy(out_sb[:, c, jb, :],
                                              ps2[:, :OUT_W])
                dst = out[s].rearrange("c (b p) i -> p c b i", p=112)
                nc.sync.dma_start(dst, out_sb[:])
    nc.compile()
    return nc


def kernel(x, _trace=False):
    global LAST_EXEC_NS, LAST_RESULTS
    from concourse.bass_utils import run_bass_kernel_spmd

    x = np.ascontiguousarray(np.asarray(x), dtype=np.float32)
    assert x.shape == (B_FULL, C, H, W + 1), x.shape

    slot_params, in_maps, assign = _prepare(x)
    key = (slot_params, USE_F32R, USE_BF16)
    if key not in _NC_CACHE:
        _NC_CACHE[key] = _build_nc(slot_params)
    nc = _NC_CACHE[key]

    res = run_bass_kernel_spmd(nc, in_maps, list(range(N_CORES)), trace=_trace)
    LAST_EXEC_NS = res.exec_time_ns
    LAST_RESULTS = res

    out_full = np.empty((B_FULL, C, OUT_H, OUT_W), np.float32)
    for s in range(B_LOC):
        for c in range(N_CORES):
            out_full[assign[s][c]] = res.results[c]["out"][s]
    return out_full



# revision 5
# speedup vs baseline: 1.7484x; 1.7484x over previous
"""Trainium2 Bass kernel for nn_CenterCrop: per-sample resize(short-side=256)
+ center-crop(224), bilinear, batch sharded over 8 NeuronCores.

Bilinear resize is separable: out = S^T @ img @ G with per-sample sparse
interpolation matrices S (vertical) and G (horizontal), built on the host
from the h/w metadata. The gather+lerp itself runs on the PE array as fp32
matmuls (exact):
  pass1: tmp1_T[x, j] = sum_y img[y, x] * S[y, j]   (img tiles stationary)
  pass2: out[j, i]    = sum_x tmp1_T[x, j] * G[x, i] (tmp1 tiles stationary)

Perf structure:
- Only the per-sample source window [y0min:y1max, x0min:x1max] that the
  output actually reads (~0.875*min(h,w) squared, 35-80% of the image) is
  DMA'd and processed.
- S/G are banded: each 128-row K-tile only touches a narrow output column
  range, so every matmul streams just that band (PSUM has_written bits make
  split accumulation exact).
- SPMD requires one program for all 8 cores, so samples are sorted by
  min(h,w) and dealt round-robin: slot s on every core holds same-sized
  windows; the program is specialized per-slot to the union shape/bands.
  Outputs are unpermuted on the host.

Measured on trn2 (8 cores): HW exec 115.2 us, max rel err 1.6e-7
(naive full-image fp32 matmul version: 245 us). PE-bound at the fp32
instruction floor (~324 logical matmuls x 2 LDW+MM pairs x ~150 ns);
DMA is 20.6 MB/core (~58 us at roofline).
"""

import sys
import os

for _p in ("/opt/trn_rl_repo",):
    if os.path.isdir(_p) and _p not in sys.path:
        sys.path.insert(0, _p)

import numpy as np

OUT_H = 224
OUT_W = 224
RESIZE_TO = np.float32(256.0)
B_FULL = 64
N_CORES = 8
B_LOC = B_FULL // N_CORES  # 8 slots per core
C = 3
H = 512
W = 512  # image width after stripping the metadata column (stored width 513)

LAST_EXEC_NS = None
LAST_RESULTS = None
_NC_CACHE = {}

# float32r experiment: single-pass PE fp32 (TF32-like rate) with streams
# padded to 256 cols to hit the fast path. Gated off by default; flip via
# env CENTERCROP_F32R=1 to measure precision/speed.
USE_F32R = os.environ.get("CENTERCROP_F32R", "0") == "1"
PADN = 256  # padded stream width under f32r

# bf16 3-pass experiment: img = A+B and S = Sa+Sb as exact bf16 pairs;
# img@S ~= A@Sa + B@Sa + A@Sb (dropped B@Sb <= 2^-18) -> ~1e-5 rel err
# with 1cyc/row streams and FWL weight loads.
USE_BF16 = os.environ.get("CENTERCROP_BF16", "0") == "1"

# bf16 single-pass (default): cast img/S/G to bf16, one matmul per tile,
# fp32 PSUM accumulate. ~5e-3 rel err vs the 2e-2 gate; full-rate PE with
# FWL weight loads and half the DMA bytes of fp32.
USE_BF16_1P = (os.environ.get("CENTERCROP_F32", "0") != "1"
               and not USE_F32R and not USE_BF16)


def _interp_matrices(h, w):
    """Full S [512, OUT_H], G [512, OUT_W] fp32 interpolation matrices,
    mirroring the reference fp32 math bit-for-bit."""
    f32 = np.float32
    h = f32(h)
    w = f32(w)
    min_dim = min(h, w)
    scale = RESIZE_TO / min_dim
    h_res = np.round(h * scale)
    w_res = np.round(w * scale)
    top = np.round((h_res - f32(OUT_H)) / f32(2.0))
    left = np.round((w_res - f32(OUT_W)) / f32(2.0))

    def axis_mat(n_out, offset, dim, dim_res, n_src):
        idx = np.arange(n_out, dtype=np.float32) + offset
        src = np.clip((idx + f32(0.5)) * dim / dim_res - f32(0.5),
                      f32(0.0), dim - f32(1.0))
        p0f = np.floor(src)
        frac = src - p0f
        imax = np.int32(dim) - 1
        p0 = np.clip(p0f.astype(np.int32), 0, imax)
        p1 = np.minimum(p0 + 1, imax)
        mat = np.zeros((n_src, n_out), np.float32)
        cols = np.arange(n_out)
        np.add.at(mat, (p0, cols), f32(1.0) - frac)
        np.add.at(mat, (p1, cols), frac)
        return mat

    S = axis_mat(OUT_H, top, h, h_res, H)
    G = axis_mat(OUT_W, left, w, w_res, W)
    return S, G


def _bands(mat_w, n_tiles):
    """Per-128-row-tile [lo, hi) columns with any nonzero; None if empty."""
    out = []
    for t in range(n_tiles):
        rows = mat_w[t * 128:(t + 1) * 128]
        nz = np.nonzero(rows.any(axis=0))[0]
        out.append(None if nz.size == 0 else (int(nz[0]), int(nz[-1]) + 1))
    return out


def _union_bands(band_lists):
    n = len(band_lists[0])
    out = []
    for t in range(n):
        los = [b[t][0] for b in band_lists if b[t] is not None]
        his = [b[t][1] for b in band_lists if b[t] is not None]
        out.append(None if not los else (min(los), max(his)))
    return out


def _prepare(x):
    """Host prep: per-sample windows/matrices, sorted slot assignment,
    per-core packed inputs, and the per-slot program parameters."""
    h_all = x[:, 0, 0, -1].astype(np.float32)
    w_all = x[:, 1, 0, -1].astype(np.float32)

    samples = []
    for b in range(B_FULL):
        S, G = _interp_matrices(h_all[b], w_all[b])
        ynz = np.nonzero(S.any(axis=1))[0]
        xnz = np.nonzero(G.any(axis=1))[0]
        y0, y1 = int(ynz[0]), int(ynz[-1]) + 1
        x0, x1 = int(xnz[0]), int(xnz[-1]) + 1
        samples.append(dict(S=S[y0:y1], G=G[x0:x1], y0=y0, x0=x0,
                            wh=y1 - y0, ww=x1 - x0))

    order = np.argsort(np.minimum(h_all, w_all), kind="stable")
    # slot s, core c -> sample order[s*N_CORES + c]
    assign = [[int(order[s * N_CORES + c]) for c in range(N_CORES)]
              for s in range(B_LOC)]

    slot_params = []
    slot_data = []  # per slot: list over cores of (sid, Sw_pad, Gw_pad)
    for s in range(B_LOC):
        sids = assign[s]
        wh = max(samples[i]["wh"] for i in sids)
        ww = max(samples[i]["ww"] for i in sids)
        n_yt = (wh + 127) // 128
        n_xt = (ww + 127) // 128
        sb_list, gb_list, data = [], [], []
        for i in sids:
            sp = samples[i]
            Sw = np.zeros((n_yt * 128, OUT_H), np.float32)
            Sw[:sp["wh"]] = sp["S"]
            Gw = np.zeros((n_xt * 128, OUT_W), np.float32)
            Gw[:sp["ww"]] = sp["G"]
            sb_list.append(_bands(Sw, n_yt))
            gb_list.append(_bands(Gw, n_xt))
            data.append((i, Sw, Gw))
        sbands = _union_bands(sb_list)
        gbands = _union_bands(gb_list)
        slot_params.append((n_yt, n_xt, ww,
                            tuple(sbands), tuple(gbands)))
        slot_data.append(data)

    # pack per-core input maps
    sgw = PADN if USE_F32R else OUT_H
    in_maps = [{} for _ in range(N_CORES)]
    for s in range(B_LOC):
        n_yt, n_xt, ww, _, _ = slot_params[s]
        for c in range(N_CORES):
            sid, Sw, Gw = slot_data[s][c]
            sp = samples[sid]
            xw = np.zeros((C, n_yt * 128, ww), np.float32)
            xw[:, :sp["wh"], :sp["ww"]] = x[
                sid, :, sp["y0"]:sp["y0"] + sp["wh"],
                sp["x0"]:sp["x0"] + sp["ww"]]
            st = np.zeros((128, n_yt, sgw), np.float32)
            st[:, :, :OUT_H] = Sw.reshape(n_yt, 128, OUT_H).transpose(1, 0, 2)
            gt = np.zeros((128, n_xt, sgw), np.float32)
            gt[:, :, :OUT_W] = Gw.reshape(n_xt, 128, OUT_W).transpose(1, 0, 2)
            if USE_BF16:
                import ml_dtypes
                bf16 = ml_dtypes.bfloat16
                for nm, arr in (("xw", xw), ("s", st), ("g", gt)):
                    a = arr.astype(bf16)
                    b = (arr - a.astype(np.float32)).astype(bf16)
                    in_maps[c][f"{nm}a{s}"] = a
                    in_maps[c][f"{nm}b{s}"] = b
            elif USE_BF16_1P:
                import ml_dtypes
                bf16 = ml_dtypes.bfloat16
                in_maps[c][f"xw{s}"] = xw.astype(bf16)
                in_maps[c][f"s{s}"] = st.astype(bf16)
                in_maps[c][f"g{s}"] = gt.astype(bf16)
            else:
                in_maps[c][f"xw{s}"] = xw
                in_maps[c][f"s{s}"] = st
                in_maps[c][f"g{s}"] = gt
    return tuple(slot_params), in_maps, assign


def _build_nc(slot_params):
    import concourse.bacc as bacc
    import concourse.mybir as mybir
    import concourse.tile as tile

    dt = mybir.dt.float32
    dtd = mybir.dt.float32r if USE_F32R else mybir.dt.float32
    sgw = PADN if USE_F32R else OUT_H
    nc = bacc.Bacc(
        "TRN2",
        target_bir_lowering=False,
        debug=False,
        enable_asserts=False,
        num_devices=N_CORES,
    )
    dtb = mybir.dt.bfloat16
    if USE_BF16 or USE_BF16_1P:
        dtd = dtb
    xw_in, s_in, g_in = [], [], []
    for s, (n_yt, n_xt, ww, _, _) in enumerate(slot_params):
        if USE_BF16:
            xw_in.append(tuple(
                nc.dram_tensor(f"xw{h}{s}", [C, n_yt * 128, ww], dtb,
                               kind="ExternalInput") for h in "ab"))
            s_in.append(tuple(
                nc.dram_tensor(f"s{h}{s}", [128, n_yt, sgw], dtb,
                               kind="ExternalInput") for h in "ab"))
            g_in.append(tuple(
                nc.dram_tensor(f"g{h}{s}", [128, n_xt, sgw], dtb,
                               kind="ExternalInput") for h in "ab"))
        else:
            xw_in.append(nc.dram_tensor(f"xw{s}", [C, n_yt * 128, ww], dtd,
                                        kind="ExternalInput"))
            s_in.append(nc.dram_tensor(f"s{s}", [128, n_yt, sgw], dtd,
                                       kind="ExternalInput"))
            g_in.append(nc.dram_tensor(f"g{s}", [128, n_xt, sgw], dtd,
                                       kind="ExternalInput"))
    out = nc.dram_tensor("out", [B_LOC, C, OUT_H, OUT_W], dt,
                         kind="ExternalOutput")

    # ascending size: the first compute wave waits on the smallest DMA
    slot_order = list(range(len(slot_params)))

    with tile.TileContext(nc) as tc:
        with (
            tc.tile_pool(name="img", bufs=4) as img_pool,
            tc.tile_pool(name="sg", bufs=3) as sg_pool,
            tc.tile_pool(name="tmp", bufs=2) as tmp_pool,
            tc.tile_pool(name="outp", bufs=2) as out_pool,
            tc.tile_pool(name="ps1", bufs=3, space="PSUM") as ps1_pool,
            tc.tile_pool(name="ps2", bufs=3, space="PSUM") as ps2_pool,
        ):
            for s in slot_order:
                n_yt, n_xt, ww, sbands, gbands = slot_params[s]
                if USE_BF16:
                    s_sb = [sg_pool.tile([128, n_yt, sgw], dtb, tag=f"s{h}",
                                          name=f"s{h}_sb")
                            for h in "ab"]
                    g_sb = [sg_pool.tile([128, n_xt, sgw], dtb, tag=f"g{h}",
                                          name=f"g{h}_sb")
                            for h in "ab"]
                    for h in range(2):
                        nc.sync.dma_start(s_sb[h][:], s_in[s][h][:])
                        nc.sync.dma_start(g_sb[h][:], g_in[s][h][:])
                else:
                    s_sb = sg_pool.tile([128, n_yt, sgw], dtd, tag="s")
                    g_sb = sg_pool.tile([128, n_xt, sgw], dtd, tag="g")
                    nc.sync.dma_start(s_sb[:], s_in[s][:])
                    nc.sync.dma_start(g_sb[:], g_in[s][:])
                out_sb = out_pool.tile([112, C, 2, OUT_W], dt)
                s_emit = [t for t in range(n_yt) if sbands[t] is not None]
                g_emit = [t for t in range(n_xt) if gbands[t] is not None]
                for c in range(C):
                    if USE_BF16:
                        img_sb = [img_pool.tile([128, n_yt, ww], dtb, tag=f"img{h}",
                                                name=f"img{h}_sb")
                                  for h in "ab"]
                        for h in range(2):
                            src = xw_in[s][h][c].rearrange(
                                "(t p) x -> p t x", p=128)
                            nc.sync.dma_start(img_sb[h][:], src)
                        tmp_sb = [tmp_pool.tile([128, n_xt, OUT_H], dtb, tag=f"tmp{h}",
                                                name=f"tmp{h}_sb")
                                  for h in "ab"]
                    else:
                        img_sb = img_pool.tile([128, n_yt, ww], dtd)
                        src = xw_in[s][c].rearrange("(t p) x -> p t x", p=128)
                        nc.sync.dma_start(img_sb[:], src)
                        tmp_sb = tmp_pool.tile([128, n_xt, OUT_H], dtd)
                    for xb in range(n_xt):
                        xlo = xb * 128
                        xn = min(128, ww - xlo)
                        ps1 = ps1_pool.tile([128, sgw], dt)
                        if USE_BF16:
                            n3 = 3 * len(s_emit)
                            k = 0
                            for t in s_emit:
                                lo, hi = sbands[t]
                                for iw, sw in ((0, 0), (1, 0), (0, 1)):
                                    nc.tensor.matmul(
                                        ps1[:xn, lo:hi],
                                        img_sb[iw][:, t, xlo:xlo + xn],
                                        s_sb[sw][:, t, lo:hi],
                                        start=(k == 0),
                                        stop=(k == n3 - 1),
                                        skip_group_check=True,
                                    )
                                    k += 1
                            nc.vector.tensor_copy(tmp_sb[0][:xn, xb, :],
                                                  ps1[:xn, :OUT_H])
                            nc.vector.tensor_sub(tmp_sb[1][:xn, xb, :],
                                                 ps1[:xn, :OUT_H],
                                                 tmp_sb[0][:xn, xb, :])
                        else:
                            for i_t, t in enumerate(s_emit):
                                lo, hi = (0, sgw) if USE_F32R else sbands[t]
                                nc.tensor.matmul(
                                    ps1[:xn, lo:hi],
                                    img_sb[:, t, xlo:xlo + xn],
                                    s_sb[:, t, lo:hi],
                                    start=(i_t == 0),
                                    stop=(i_t == len(s_emit) - 1),
                                    skip_group_check=True,
                                )
                            nc.vector.tensor_copy(tmp_sb[:xn, xb, :],
                                                  ps1[:xn, :OUT_H])
                    for jb in range(2):
                        ps2 = ps2_pool.tile([112, sgw], dt)
                        if USE_BF16:
                            n3 = 3 * len(g_emit)
                            k = 0
                            for xb in g_emit:
                                lo, hi = gbands[xb]
                                xn = min(128, ww - xb * 128)
                                for tw, gw in ((0, 0), (1, 0), (0, 1)):
                                    nc.tensor.matmul(
                                        ps2[:, lo:hi],
                                        tmp_sb[tw][:xn, xb,
                                                   jb * 112:(jb + 1) * 112],
                                        g_sb[gw][:xn, xb, lo:hi],
                                        start=(k == 0),
                                        stop=(k == n3 - 1),
                                        skip_group_check=True,
                                    )
                                    k += 1
                        else:
                            for i_t, xb in enumerate(g_emit):
                                lo, hi = (0, sgw) if USE_F32R else gbands[xb]
                                xn = min(128, ww - xb * 128)
                                nc.tensor.matmul(
                                    ps2[:, lo:hi],
                                    tmp_sb[:xn, xb, jb * 112:(jb + 1) * 112],
                                    g_sb[:xn, xb, lo:hi],
                                    start=(i_t == 0),
                                    stop=(i_t == len(g_emit) - 1),
                                    skip_group_check=True,
                                )
                        nc.vector.tensor_copy(out_sb[:, c, jb, :],
                                              ps2[:, :OUT_W])
                dst = out[s].rearrange("c (b p) i -> p c b i", p=112)
                nc.sync.dma_start(dst, out_sb[:])
    nc.compile()
    return nc


def kernel(x, _trace=False):
    global LAST_EXEC_NS, LAST_RESULTS
    from concourse.bass_utils import run_bass_kernel_spmd

    x = np.ascontiguousarray(np.asarray(x), dtype=np.float32)
    assert x.shape == (B_FULL, C, H, W + 1), x.shape

    slot_params, in_maps, assign = _prepare(x)
    key = (slot_params, USE_F32R, USE_BF16, USE_BF16_1P)
    if key not in _NC_CACHE:
        _NC_CACHE[key] = _build_nc(slot_params)
    nc = _NC_CACHE[key]

    res = run_bass_kernel_spmd(nc, in_maps, list(range(N_CORES)), trace=_trace)
    LAST_EXEC_NS = res.exec_time_ns
    LAST_RESULTS = res

    out_full = np.empty((B_FULL, C, OUT_H, OUT_W), np.float32)
    for s in range(B_LOC):
        for c in range(N_CORES):
            out_full[assign[s][c]] = res.results[c]["out"][s]
    return out_full



# revision 6
# speedup vs baseline: 1.9607x; 1.1214x over previous
"""Trainium2 Bass kernel for nn_CenterCrop: per-sample resize(short-side=256)
+ center-crop(224), bilinear, batch sharded over 8 NeuronCores.

Bilinear resize is separable: out = S^T @ img @ G with per-sample sparse
interpolation matrices S (vertical) and G (horizontal), built on the host
from the h/w metadata. The gather+lerp runs on the PE array as bf16 matmuls
with fp32 PSUM accumulation (rel err ~7e-3 vs the 2e-2 gate):
  pass1: tmp1_T[x, j] = sum_y img[y, x] * S[y, j]   (img tiles stationary)
  pass2: out[j, i]    = sum_x tmp1_T[x, j] * G[x, i] (tmp1 tiles stationary)

Perf structure:
- Only the per-sample source window [y0min:y1max, x0min:x1max] that the
  output actually reads (~0.875*min(h,w) squared) is DMA'd and processed.
- S/G are banded: each 128-row K-tile only touches a narrow output column
  range; only those bands are packed/DMA'd/streamed (PSUM has_written bits
  make split accumulation exact).
- All inputs for one slot (S bands + G bands + 3 channel windows) are
  host-packed into ONE [128, TOT] bf16 tensor laid out partition-major, so
  each slot is a single linear DMA with ~TOT*2 B contiguous per partition.
- Output is written as bf16 in SBUF layout [112, C, 2jb, 224] (linear DMA);
  the host unpermutes and upcasts to fp32.
- SPMD requires one program for all 8 cores, so samples are sorted by
  min(h,w) and dealt round-robin: slot s on every core holds same-sized
  windows; the program is specialized per-slot to the union shape/bands.

History (HW, 8 cores): fp32 exact 117.8us -> bf16 single-pass 67.4us ->
this packed-DMA/bf16-out version.
"""

import sys
import os

for _p in ("/opt/trn_rl_repo",):
    if os.path.isdir(_p) and _p not in sys.path:
        sys.path.insert(0, _p)

import numpy as np
import ml_dtypes

BF16 = ml_dtypes.bfloat16

OUT_H = 224
OUT_W = 224
RESIZE_TO = np.float32(256.0)
B_FULL = 64
N_CORES = 8
B_LOC = B_FULL // N_CORES  # 8 slots per core
C = 3
H = 512
W = 512  # image width after stripping the metadata column (stored width 513)

LAST_EXEC_NS = None
LAST_RESULTS = None
_NC_CACHE = {}


def _interp_matrices(h, w):
    """Full S [512, OUT_H], G [512, OUT_W] fp32 interpolation matrices,
    mirroring the reference fp32 math bit-for-bit."""
    f32 = np.float32
    h = f32(h)
    w = f32(w)
    min_dim = min(h, w)
    scale = RESIZE_TO / min_dim
    h_res = np.round(h * scale)
    w_res = np.round(w * scale)
    top = np.round((h_res - f32(OUT_H)) / f32(2.0))
    left = np.round((w_res - f32(OUT_W)) / f32(2.0))

    def axis_mat(n_out, offset, dim, dim_res, n_src):
        idx = np.arange(n_out, dtype=np.float32) + offset
        src = np.clip((idx + f32(0.5)) * dim / dim_res - f32(0.5),
                      f32(0.0), dim - f32(1.0))
        p0f = np.floor(src)
        frac = src - p0f
        imax = np.int32(dim) - 1
        p0 = np.clip(p0f.astype(np.int32), 0, imax)
        p1 = np.minimum(p0 + 1, imax)
        mat = np.zeros((n_src, n_out), np.float32)
        cols = np.arange(n_out)
        np.add.at(mat, (p0, cols), f32(1.0) - frac)
        np.add.at(mat, (p1, cols), frac)
        return mat

    S = axis_mat(OUT_H, top, h, h_res, H)
    G = axis_mat(OUT_W, left, w, w_res, W)
    return S, G


def _bands(mat_w, n_tiles):
    """Per-128-row-tile [lo, hi) columns with any nonzero; None if empty."""
    out = []
    for t in range(n_tiles):
        rows = mat_w[t * 128:(t + 1) * 128]
        nz = np.nonzero(rows.any(axis=0))[0]
        out.append(None if nz.size == 0 else (int(nz[0]), int(nz[-1]) + 1))
    return out


def _union_bands(band_lists):
    n = len(band_lists[0])
    out = []
    for t in range(n):
        los = [b[t][0] for b in band_lists if b[t] is not None]
        his = [b[t][1] for b in band_lists if b[t] is not None]
        out.append(None if not los else (min(los), max(his)))
    return out


def _offsets(bands):
    """Packed offsets for the non-empty bands; returns (offs, total)."""
    offs = []
    tot = 0
    for b in bands:
        if b is None:
            offs.append(None)
        else:
            offs.append(tot)
            tot += b[1] - b[0]
    return tuple(offs), tot


def _prepare(x):
    """Host prep: per-sample windows/matrices, sorted slot assignment,
    per-core packed single-tensor inputs, and per-slot program params."""
    h_all = x[:, 0, 0, -1].astype(np.float32)
    w_all = x[:, 1, 0, -1].astype(np.float32)

    samples = []
    for b in range(B_FULL):
        S, G = _interp_matrices(h_all[b], w_all[b])
        ynz = np.nonzero(S.any(axis=1))[0]
        xnz = np.nonzero(G.any(axis=1))[0]
        y0, y1 = int(ynz[0]), int(ynz[-1]) + 1
        x0, x1 = int(xnz[0]), int(xnz[-1]) + 1
        samples.append(dict(S=S[y0:y1], G=G[x0:x1], y0=y0, x0=x0,
                            wh=y1 - y0, ww=x1 - x0))

    order = np.argsort(np.minimum(h_all, w_all), kind="stable")
    # slot s, core c -> sample order[s*N_CORES + c]
    assign = [[int(order[s * N_CORES + c]) for c in range(N_CORES)]
              for s in range(B_LOC)]

    slot_params = []
    in_maps = [{} for _ in range(N_CORES)]
    for s in range(B_LOC):
        sids = assign[s]
        wh = max(samples[i]["wh"] for i in sids)
        ww = max(samples[i]["ww"] for i in sids)
        n_yt = (wh + 127) // 128
        n_xt = (ww + 127) // 128
        sb_list, gb_list = [], []
        for i in sids:
            sp = samples[i]
            Sw = np.zeros((n_yt * 128, OUT_H), np.float32)
            Sw[:sp["wh"]] = sp["S"]
            Gw = np.zeros((n_xt * 128, OUT_W), np.float32)
            Gw[:sp["ww"]] = sp["G"]
            sb_list.append(_bands(Sw, n_yt))
            gb_list.append(_bands(Gw, n_xt))
        sbands = tuple(_union_bands(sb_list))
        gbands = tuple(_union_bands(gb_list))
        s_offs, s_tot = _offsets(sbands)
        g_offs, g_tot = _offsets(gbands)
        img_off = s_tot + g_tot
        tot = img_off + C * n_yt * ww
        slot_params.append((n_yt, n_xt, ww, sbands, gbands,
                            s_offs, g_offs, s_tot, tot))

        for c_core in range(N_CORES):
            sid = sids[c_core]
            sp = samples[sid]
            Sw = np.zeros((n_yt * 128, OUT_H), np.float32)
            Sw[:sp["wh"]] = sp["S"]
            Gw = np.zeros((n_xt * 128, OUT_W), np.float32)
            Gw[:sp["ww"]] = sp["G"]
            arr = np.zeros((128, tot), BF16)
            # S bands: partition p, tile t -> S row t*128+p, cols [lo, hi)
            for t in range(n_yt):
                if sbands[t] is None:
                    continue
                lo, hi = sbands[t]
                off = s_offs[t]
                arr[:, off:off + hi - lo] = Sw[t * 128:(t + 1) * 128, lo:hi]
            for t in range(n_xt):
                if gbands[t] is None:
                    continue
                lo, hi = gbands[t]
                off = s_tot + g_offs[t]
                arr[:, off:off + hi - lo] = Gw[t * 128:(t + 1) * 128, lo:hi]
            # image windows, partition-major: arr[p, img_off + (c*n_yt+t)*ww + x]
            xw = np.zeros((C, n_yt * 128, ww), np.float32)
            xw[:, :sp["wh"], :sp["ww"]] = x[
                sid, :, sp["y0"]:sp["y0"] + sp["wh"],
                sp["x0"]:sp["x0"] + sp["ww"]]
            # [C, n_yt, 128, ww] -> [128, C, n_yt, ww]
            arr[:, img_off:] = xw.reshape(C, n_yt, 128, ww).transpose(
                2, 0, 1, 3).reshape(128, C * n_yt * ww)
            in_maps[c_core][f"in{s}"] = arr
    return tuple(slot_params), in_maps, assign


def _build_nc(slot_params):
    import concourse.bacc as bacc
    import concourse.mybir as mybir
    import concourse.tile as tile

    dt = mybir.dt.float32
    dtb = mybir.dt.bfloat16
    nc = bacc.Bacc(
        "TRN2",
        target_bir_lowering=False,
        debug=False,
        enable_asserts=False,
        num_devices=N_CORES,
    )
    in_dram = [nc.dram_tensor(f"in{s}", [128, p[-1]], dtb,
                              kind="ExternalInput")
               for s, p in enumerate(slot_params)]
    out = nc.dram_tensor("out", [B_LOC, 112, C, 2, OUT_W], dtb,
                         kind="ExternalOutput")

    with tile.TileContext(nc) as tc:
        with (
            tc.tile_pool(name="inp", bufs=3) as in_pool,
            tc.tile_pool(name="tmp", bufs=2) as tmp_pool,
            tc.tile_pool(name="outp", bufs=2) as out_pool,
            tc.tile_pool(name="ps1", bufs=3, space="PSUM") as ps1_pool,
            tc.tile_pool(name="ps2", bufs=3, space="PSUM") as ps2_pool,
        ):
            for s, (n_yt, n_xt, ww, sbands, gbands,
                    s_offs, g_offs, s_tot, tot) in enumerate(slot_params):
                in_sb = in_pool.tile([128, tot], dtb)
                nc.sync.dma_start(in_sb[:], in_dram[s][:])
                img_off = s_tot + sum(
                    b[1] - b[0] for b in gbands if b is not None)
                out_sb = out_pool.tile([112, C, 2, OUT_W], dtb)
                s_emit = [t for t in range(n_yt) if sbands[t] is not None]
                g_emit = [t for t in range(n_xt) if gbands[t] is not None]
                for c in range(C):
                    tmp_sb = tmp_pool.tile([128, n_xt, OUT_H], dtb)
                    for xb in range(n_xt):
                        xlo = xb * 128
                        xn = min(128, ww - xlo)
                        ps1 = ps1_pool.tile([128, OUT_H], dt)
                        for i_t, t in enumerate(s_emit):
                            lo, hi = sbands[t]
                            ib = img_off + (c * n_yt + t) * ww + xlo
                            so = s_offs[t]
                            nc.tensor.matmul(
                                ps1[:xn, lo:hi],
                                in_sb[:, ib:ib + xn],
                                in_sb[:, so:so + hi - lo],
                                start=(i_t == 0),
                                stop=(i_t == len(s_emit) - 1),
                                skip_group_check=True,
                            )
                        nc.vector.tensor_copy(tmp_sb[:xn, xb, :],
                                              ps1[:xn, :OUT_H])
                    for jb in range(2):
                        ps2 = ps2_pool.tile([112, OUT_W], dt)
                        for i_t, xb in enumerate(g_emit):
                            lo, hi = gbands[xb]
                            xn = min(128, ww - xb * 128)
                            go = s_tot + g_offs[xb]
                            nc.tensor.matmul(
                                ps2[:, lo:hi],
                                tmp_sb[:xn, xb, jb * 112:(jb + 1) * 112],
                                in_sb[:xn, go:go + hi - lo],
                                start=(i_t == 0),
                                stop=(i_t == len(g_emit) - 1),
                                skip_group_check=True,
                            )
                        nc.vector.tensor_copy(out_sb[:, c, jb, :],
                                              ps2[:, :OUT_W])
                nc.sync.dma_start(out[s], out_sb[:])
    nc.compile()
    return nc


def kernel(x, _trace=False):
    global LAST_EXEC_NS, LAST_RESULTS
    from concourse.bass_utils import run_bass_kernel_spmd

    x = np.ascontiguousarray(np.asarray(x), dtype=np.float32)
    assert x.shape == (B_FULL, C, H, W + 1), x.shape

    slot_params, in_maps, assign = _prepare(x)
    key = slot_params
    if key not in _NC_CACHE:
        _NC_CACHE[key] = _build_nc(slot_params)
    nc = _NC_CACHE[key]

    res = run_bass_kernel_spmd(nc, in_maps, list(range(N_CORES)), trace=_trace)
    LAST_EXEC_NS = res.exec_time_ns
    LAST_RESULTS = res

    out_full = np.empty((B_FULL, C, OUT_H, OUT_W), np.float32)
    for s in range(B_LOC):
        for c in range(N_CORES):
            # [112, C, 2, 224] -> [C, 2, 112, 224] -> [C, 224, 224]
            arr = np.asarray(res.results[c]["out"][s]).astype(np.float32)
            out_full[assign[s][c]] = arr.transpose(1, 2, 0, 3).reshape(
                C, OUT_H, OUT_W)
    return out_full


# revision 7
# speedup vs baseline: 1.9709x; 1.0052x over previous
"""Trainium2 Bass kernel for nn_CenterCrop: per-sample resize(short-side=256)
+ center-crop(224), bilinear, batch sharded over 8 NeuronCores.

Bilinear resize is separable: out = S^T @ img @ G with per-sample sparse
interpolation matrices S (vertical) and G (horizontal), built on the host
from the h/w metadata. The gather+lerp runs on the PE array as bf16 matmuls
with fp32 PSUM accumulation (rel err ~7e-3 vs the 2e-2 gate):
  pass1: tmp1_T[x, j] = sum_y img[y, x] * S[y, j]   (img tiles stationary)
  pass2: out[j, i]    = sum_x tmp1_T[x, j] * G[x, i] (tmp1 tiles stationary)

Perf structure:
- Only the per-sample source window that the output reads (~0.875*min(h,w)
  squared) is DMA'd/processed; S/G are banded and only bands are packed,
  DMA'd, and streamed (PSUM has_written bits make split accumulation exact).
- Inputs per slot are packed into TWO [128, *] bf16 tensors laid out
  partition-major (one linear DMA each): A = S bands + channel-0 window,
  B = G bands + channel-1/2 windows. pass1 c0 only waits on A, shrinking
  the initial fill; A frees after pass1.
- PSUM->SBUF drains run on the Scalar engine (ACT Copy, 172+FD cyc, idle
  otherwise); the Vector engine only does the pass1 tmp casts. Output is
  bf16 [112, 2, 224] per (slot, channel), DMA'd per channel, host unpermutes
  and upcasts.
- SPMD requires one program for all 8 cores, so samples are sorted by
  min(h,w) and dealt round-robin: slot s on every core holds same-sized
  windows; the program is specialized per-slot to the union shape/bands.
  Slot order small-first (fast fill) and small-last (fast tail).

History (HW, 8 cores): fp32 exact 117.8us -> bf16 single-pass 67.4us ->
packed single-DMA inputs + bf16 out 60.1us -> this version.
"""

import sys
import os

for _p in ("/opt/trn_rl_repo",):
    if os.path.isdir(_p) and _p not in sys.path:
        sys.path.insert(0, _p)

import numpy as np
import ml_dtypes

BF16 = ml_dtypes.bfloat16

OUT_H = 224
OUT_W = 224
RESIZE_TO = np.float32(256.0)
B_FULL = 64
N_CORES = 8
B_LOC = B_FULL // N_CORES  # 8 slots per core
C = 3
H = 512
W = 512  # image width after stripping the metadata column (stored width 513)

LAST_EXEC_NS = None
LAST_RESULTS = None
_NC_CACHE = {}


def _interp_matrices(h, w):
    """Full S [512, OUT_H], G [512, OUT_W] fp32 interpolation matrices,
    mirroring the reference fp32 math bit-for-bit."""
    f32 = np.float32
    h = f32(h)
    w = f32(w)
    min_dim = min(h, w)
    scale = RESIZE_TO / min_dim
    h_res = np.round(h * scale)
    w_res = np.round(w * scale)
    top = np.round((h_res - f32(OUT_H)) / f32(2.0))
    left = np.round((w_res - f32(OUT_W)) / f32(2.0))

    def axis_mat(n_out, offset, dim, dim_res, n_src):
        idx = np.arange(n_out, dtype=np.float32) + offset
        src = np.clip((idx + f32(0.5)) * dim / dim_res - f32(0.5),
                      f32(0.0), dim - f32(1.0))
        p0f = np.floor(src)
        frac = src - p0f
        imax = np.int32(dim) - 1
        p0 = np.clip(p0f.astype(np.int32), 0, imax)
        p1 = np.minimum(p0 + 1, imax)
        mat = np.zeros((n_src, n_out), np.float32)
        cols = np.arange(n_out)
        np.add.at(mat, (p0, cols), f32(1.0) - frac)
        np.add.at(mat, (p1, cols), frac)
        return mat

    S = axis_mat(OUT_H, top, h, h_res, H)
    G = axis_mat(OUT_W, left, w, w_res, W)
    return S, G


def _bands(mat_w, n_tiles):
    """Per-128-row-tile [lo, hi) columns with any nonzero; None if empty."""
    out = []
    for t in range(n_tiles):
        rows = mat_w[t * 128:(t + 1) * 128]
        nz = np.nonzero(rows.any(axis=0))[0]
        out.append(None if nz.size == 0 else (int(nz[0]), int(nz[-1]) + 1))
    return out


def _union_bands(band_lists):
    n = len(band_lists[0])
    out = []
    for t in range(n):
        los = [b[t][0] for b in band_lists if b[t] is not None]
        his = [b[t][1] for b in band_lists if b[t] is not None]
        out.append(None if not los else (min(los), max(his)))
    return out


def _offsets(bands):
    """Packed offsets for the non-empty bands; returns (offs, total)."""
    offs = []
    tot = 0
    for b in bands:
        if b is None:
            offs.append(None)
        else:
            offs.append(tot)
            tot += b[1] - b[0]
    return tuple(offs), tot


def _prepare(x):
    """Host prep: per-sample windows/matrices, sorted slot assignment,
    per-core packed A/B tensors, and per-slot program params."""
    h_all = x[:, 0, 0, -1].astype(np.float32)
    w_all = x[:, 1, 0, -1].astype(np.float32)

    samples = []
    for b in range(B_FULL):
        S, G = _interp_matrices(h_all[b], w_all[b])
        ynz = np.nonzero(S.any(axis=1))[0]
        xnz = np.nonzero(G.any(axis=1))[0]
        y0, y1 = int(ynz[0]), int(ynz[-1]) + 1
        x0, x1 = int(xnz[0]), int(xnz[-1]) + 1
        samples.append(dict(S=S[y0:y1], G=G[x0:x1], y0=y0, x0=x0,
                            wh=y1 - y0, ww=x1 - x0))

    order = np.argsort(np.minimum(h_all, w_all), kind="stable")
    # slot s, core c -> sample order[s*N_CORES + c]
    assign = [[int(order[s * N_CORES + c]) for c in range(N_CORES)]
              for s in range(B_LOC)]

    slot_params = []
    in_maps = [{} for _ in range(N_CORES)]
    for s in range(B_LOC):
        sids = assign[s]
        wh = max(samples[i]["wh"] for i in sids)
        ww = max(samples[i]["ww"] for i in sids)
        n_yt = (wh + 127) // 128
        n_xt = (ww + 127) // 128
        sb_list, gb_list = [], []
        for i in sids:
            sp = samples[i]
            Sw = np.zeros((n_yt * 128, OUT_H), np.float32)
            Sw[:sp["wh"]] = sp["S"]
            Gw = np.zeros((n_xt * 128, OUT_W), np.float32)
            Gw[:sp["ww"]] = sp["G"]
            sb_list.append(_bands(Sw, n_yt))
            gb_list.append(_bands(Gw, n_xt))
        sbands = tuple(_union_bands(sb_list))
        gbands = tuple(_union_bands(gb_list))
        s_offs, s_tot = _offsets(sbands)
        g_offs, g_tot = _offsets(gbands)
        cw = n_yt * ww  # one channel's window elems per partition
        tot_a = s_tot + cw
        tot_b = g_tot + 2 * cw
        slot_params.append((n_yt, n_xt, ww, sbands, gbands,
                            s_offs, g_offs, s_tot, g_tot, tot_a, tot_b))

        for c_core in range(N_CORES):
            sid = sids[c_core]
            sp = samples[sid]
            Sw = np.zeros((n_yt * 128, OUT_H), np.float32)
            Sw[:sp["wh"]] = sp["S"]
            Gw = np.zeros((n_xt * 128, OUT_W), np.float32)
            Gw[:sp["ww"]] = sp["G"]
            arr_a = np.zeros((128, tot_a), BF16)
            arr_b = np.zeros((128, tot_b), BF16)
            for t in range(n_yt):
                if sbands[t] is None:
                    continue
                lo, hi = sbands[t]
                off = s_offs[t]
                arr_a[:, off:off + hi - lo] = Sw[t * 128:(t + 1) * 128, lo:hi]
            for t in range(n_xt):
                if gbands[t] is None:
                    continue
                lo, hi = gbands[t]
                off = g_offs[t]
                arr_b[:, off:off + hi - lo] = Gw[t * 128:(t + 1) * 128, lo:hi]
            # image windows, partition-major: row t*128+p -> [p, t*ww + x]
            xw = np.zeros((C, n_yt * 128, ww), np.float32)
            xw[:, :sp["wh"], :sp["ww"]] = x[
                sid, :, sp["y0"]:sp["y0"] + sp["wh"],
                sp["x0"]:sp["x0"] + sp["ww"]]
            xw_t = xw.reshape(C, n_yt, 128, ww).transpose(2, 0, 1, 3)
            arr_a[:, s_tot:] = xw_t[:, 0].reshape(128, cw)
            arr_b[:, g_tot:] = xw_t[:, 1:].reshape(128, 2 * cw)
            in_maps[c_core][f"ina{s}"] = arr_a
            in_maps[c_core][f"inb{s}"] = arr_b
    return tuple(slot_params), in_maps, assign


def _build_nc(slot_params):
    import concourse.bacc as bacc
    import concourse.mybir as mybir
    import concourse.tile as tile

    dt = mybir.dt.float32
    dtb = mybir.dt.bfloat16
    act_copy = mybir.ActivationFunctionType.Copy
    nc = bacc.Bacc(
        "TRN2",
        target_bir_lowering=False,
        debug=False,
        enable_asserts=False,
        num_devices=N_CORES,
    )
    in_a = [nc.dram_tensor(f"ina{s}", [128, p[-2]], dtb, kind="ExternalInput")
            for s, p in enumerate(slot_params)]
    in_b = [nc.dram_tensor(f"inb{s}", [128, p[-1]], dtb, kind="ExternalInput")
            for s, p in enumerate(slot_params)]
    out = nc.dram_tensor("out", [B_LOC, 112, C, 2, OUT_W], dtb,
                         kind="ExternalOutput")

    # small first (fast fill), 2nd-smallest last (fast drain)
    slot_order = [0] + list(range(2, B_LOC)) + [1]

    with tile.TileContext(nc) as tc:
        with (
            tc.tile_pool(name="ina", bufs=2) as a_pool,
            tc.tile_pool(name="inb", bufs=2) as b_pool,
            tc.tile_pool(name="tmp", bufs=2) as tmp_pool,
            tc.tile_pool(name="outp", bufs=4) as out_pool,
            tc.tile_pool(name="ps1", bufs=3, space="PSUM") as ps1_pool,
            tc.tile_pool(name="ps2", bufs=2, space="PSUM") as ps2_pool,
        ):
            for s in slot_order:
                (n_yt, n_xt, ww, sbands, gbands, s_offs, g_offs,
                 s_tot, g_tot, tot_a, tot_b) = slot_params[s]
                a_sb = a_pool.tile([128, tot_a], dtb)
                nc.sync.dma_start(a_sb[:], in_a[s][:])
                b_sb = b_pool.tile([128, tot_b], dtb)
                nc.sync.dma_start(b_sb[:], in_b[s][:])
                cw = n_yt * ww
                s_emit = [t for t in range(n_yt) if sbands[t] is not None]
                g_emit = [t for t in range(n_xt) if gbands[t] is not None]
                for c in range(C):
                    tmp_sb = tmp_pool.tile([128, n_xt, OUT_H], dtb)
                    for xb in range(n_xt):
                        xlo = xb * 128
                        xn = min(128, ww - xlo)
                        ps1 = ps1_pool.tile([128, OUT_H], dt)
                        for i_t, t in enumerate(s_emit):
                            lo, hi = sbands[t]
                            if c == 0:
                                img = a_sb[:, s_tot + t * ww + xlo:
                                           s_tot + t * ww + xlo + xn]
                            else:
                                ib = g_tot + ((c - 1) * n_yt + t) * ww + xlo
                                img = b_sb[:, ib:ib + xn]
                            so = s_offs[t]
                            nc.tensor.matmul(
                                ps1[:xn, lo:hi],
                                img,
                                a_sb[:, so:so + hi - lo],
                                start=(i_t == 0),
                                stop=(i_t == len(s_emit) - 1),
                                skip_group_check=True,
                            )
                        nc.vector.tensor_copy(tmp_sb[:xn, xb, :],
                                              ps1[:xn, :OUT_H])
                    ps2 = ps2_pool.tile([112, 2, OUT_W], dt)
                    for jb in range(2):
                        for i_t, xb in enumerate(g_emit):
                            lo, hi = gbands[xb]
                            xn = min(128, ww - xb * 128)
                            go = g_offs[xb]
                            nc.tensor.matmul(
                                ps2[:, jb, lo:hi],
                                tmp_sb[:xn, xb, jb * 112:(jb + 1) * 112],
                                b_sb[:xn, go:go + hi - lo],
                                start=(i_t == 0),
                                stop=(i_t == len(g_emit) - 1),
                                skip_group_check=True,
                            )
                    out_sb = out_pool.tile([112, 2, OUT_W], dtb)
                    nc.scalar.activation(out_sb[:], ps2[:], act_copy)
                    nc.sync.dma_start(out[s, :, c], out_sb[:])
    nc.compile()
    return nc


def kernel(x, _trace=False):
    global LAST_EXEC_NS, LAST_RESULTS
    from concourse.bass_utils import run_bass_kernel_spmd

    x = np.ascontiguousarray(np.asarray(x), dtype=np.float32)
    assert x.shape == (B_FULL, C, H, W + 1), x.shape

    slot_params, in_maps, assign = _prepare(x)
    key = slot_params
    if key not in _NC_CACHE:
        _NC_CACHE[key] = _build_nc(slot_params)
    nc = _NC_CACHE[key]

    res = run_bass_kernel_spmd(nc, in_maps, list(range(N_CORES)), trace=_trace)
    LAST_EXEC_NS = res.exec_time_ns
    LAST_RESULTS = res

    out_full = np.empty((B_FULL, C, OUT_H, OUT_W), np.float32)
    for s in range(B_LOC):
        for c in range(N_CORES):
            # [112, C, 2, 224] -> [C, 2, 112, 224] -> [C, 224, 224]
            arr = np.asarray(res.results[c]["out"][s]).astype(np.float32)
            out_full[assign[s][c]] = arr.transpose(1, 2, 0, 3).reshape(
                C, OUT_H, OUT_W)
    return out_full
